# revision 46
# baseline (speedup 1.0000x reference)
"""Trainium2 Bass kernel for nn_EntityGraphRefinement (gnn_message_passing).

Math note driving the implementation
------------------------------------
The reference computes entities via cross-attention (queries = entity
library, kv = inputs), then an entity-activity mask
    activity = attn.mean(axis=(H, S));  mask = activity > 0.1
Since softmax rows sum to exactly 1 over S, activity == 1/S == 1/512 for
every (b, m) regardless of the data, so mask == 0 identically and the
final `graph` output (graph * mask_row * mask_col) is identically zero.
The entire pairwise-MLP refinement and gating pipeline is dead code with
respect to the outputs.  The kernel therefore computes only `entities`
on-device (the full MHA), and returns exact zeros for `graph` and
`entity_mask`, which matches the reference bit-for-bit.

Sharding: data-parallel over batch B=8 across the 8 NeuronCores (one
batch element per core); all weights replicated.

Implementation notes:
- scoresT is computed in [s, (h,m)] layout so the softmax reduction over
  s becomes a ones-vector matmul (column sums), avoiding attn transposes.
- Per-head dh=32 contractions are batched 4-heads-at-a-time with a
  block-diagonal q operand so every matmul contracts a full 128
  partitions at base partition 0 (partition-offset matmuls / explicit
  tile_position are broken on this runtime).
- ctx is computed as a full [128hk, 4*64] product per hk-chunk; the
  valid diagonal blocks are extracted by partition-preserving copies
  fused with the softmax normalization multiply.
"""

import math

import numpy as np

# Problem dims (hardcoded per harness contract).
B, S, D = 8, 512, 256
M, DE, H = 64, 256, 8
DH = DE // H  # 32
N_CORES = 8
SC = S // 128  # 4 s-chunks of 128
DC = D // 128  # 2 d/hk chunks of 128

_CACHE: dict = {}

# Use float32r (full-rate fp32 PE mode) for large matmuls; plain fp32 is
# 4 cycles/row on TRN2, float32r is 1 cycle/row for moving dims >= 256.
USE_F32R = True


def _build_bass():
    """Build the single-core Bass program (SPMD across 8 cores)."""
    import concourse.bacc as bacc
    import concourse.mybir as mybir
    import concourse.tile as tile
    from concourse.masks import make_identity

    f32 = mybir.dt.float32
    f32r = mybir.dt.float32r if USE_F32R else mybir.dt.float32
    AF = mybir.ActivationFunctionType

    nc = bacc.Bacc("TRN2", target_bir_lowering=False)

    emb = nc.dram_tensor("emb", [S, D], f32, kind="ExternalInput")
    pos = nc.dram_tensor("pos", [S, D], mybir.dt.bfloat16, kind="ExternalInput")
    wq = nc.dram_tensor("wq", [D, DE], f32, kind="ExternalInput")
    wkvo = nc.dram_tensor("wkvo", [3 * D, DE], f32, kind="ExternalInput")
    lib = nc.dram_tensor("lib", [M, DE], f32, kind="ExternalInput")
    ent = nc.dram_tensor("ent", [M, DE], f32, kind="ExternalOutput")

    def _body(tc, mp, pp):
        # ---- constants ----
        ident = mp.tile([128, 128], f32)
        make_identity(nc, ident)
        ones_f = mp.tile([128, 1], f32)
        nc.gpsimd.memset(ones_f, 1.0)
        ones_col = mp.tile([128, 1], f32r)
        nc.gpsimd.tensor_copy(ones_col, ones_f)
        ones_rf = mp.tile([1, 128], f32)
        nc.gpsimd.memset(ones_rf, 1.0)
        ones_row = mp.tile([1, 128], f32r)
        nc.gpsimd.tensor_copy(ones_row, ones_rf)
        # q4 zero-init done early: keeps the Pool engine busy so its SWDGE
        # weight DMA below enqueues after the critical emb/pos transfers.
        q4_f = mp.tile([128, DC, 4 * M], f32)
        nc.gpsimd.memset(q4_f, 0.0)
        q4_sb = mp.tile([128, DC, 4 * M], f32r)
        nc.gpsimd.tensor_copy(q4_sb, q4_f)

        # ---- input loads ----
        # emb/pos chunked across the two HWDGE rings (SP + ACT) so adds and
        # transposes start as soon as the first chunk lands.
        x_sb = mp.tile([128, SC, D], f32)
        pos_sb = mp.tile([128, SC, D], mybir.dt.bfloat16)
        embv = emb[:].rearrange("(c p) d -> p c d", p=128)
        posv = pos[:].rearrange("(c p) d -> p c d", p=128)
        for half in range(2):
            sl = slice(half * 2, half * 2 + 2)
            nc.sync.dma_start(out=x_sb[:, sl, :], in_=embv[:, sl, :])
            nc.scalar.dma_start(out=pos_sb[:, sl, :], in_=posv[:, sl, :])
        wq_sb = mp.tile([128, DC, DE], f32)
        nc.scalar.dma_start(out=wq_sb, in_=wq[:].rearrange("(kc p) n -> p kc n", p=128))
        lib_sb = mp.tile([M, DE], f32)
        nc.sync.dma_start(out=lib_sb, in_=lib[:])
        # weights feeding f32r matmuls: SWDGE DMAs casting f32 -> f32r,
        # one per weight so the critical wk lands first.
        wkvo_r = mp.tile([128, 3 * DC, DE], f32r)
        wkvo_v = wkvo[:].rearrange("(w p) n -> p w n", p=128)
        for w in range(3):
            nc.gpsimd.dma_start(
                out=wkvo_r[:, w * DC : (w + 1) * DC, :],
                in_=wkvo_v[:, w * DC : (w + 1) * DC, :],
            )
        wk_r = wkvo_r[:, 0:DC, :]
        wv_r = wkvo_r[:, DC : 2 * DC, :]
        wo_r = wkvo_r[:, 2 * DC : 3 * DC, :]

        # Prefetch the ACT table set (exp_and_others covers Exp and Copy)
        # right after the ACT-ring DMA issues so the ~2.7us load overlaps
        # the input DMA transfers instead of stalling the first Copy/Exp.
        warm = mp.tile([1, 1], f32)
        nc.vector.memset(warm, 0.0)
        nc.scalar.activation(warm, warm, AF.Exp)

        # ---- libT[d, m] ----
        libT_sb = mp.tile([128, DC, M], f32)
        for dc in range(DC):
            libTp = pp.tile([128, M], f32, tag="mrg", bufs=2, name=f"libTp{dc}")
            nc.tensor.transpose(
                libTp, lib_sb[:, dc * 128 : (dc + 1) * 128], ident[:M, :M]
            )
            nc.vector.tensor_copy(libT_sb[:, dc, :], libTp)

        # ---- q4[hk, kc, 4*M]: block-diagonal scaled qT ----
        # q4[j*32+dh, kc, j*M + m] = qT[kc*128 + j*32 + dh, m] / sqrt(DH)
        for kc in range(DC):
            qTp = pp.tile([128, M], f32, tag="mrg", bufs=2, name=f"qTp{kc}")
            for dc in range(DC):
                nc.tensor.matmul(
                    qTp,
                    lhsT=wq_sb[:, dc, kc * 128 : (kc + 1) * 128],
                    rhs=libT_sb[:, dc, :],
                    start=(dc == 0),
                    stop=(dc == DC - 1),
                )
            for j in range(4):
                nc.scalar.mul(
                    q4_sb[j * 32 : (j + 1) * 32, kc, j * M : (j + 1) * M],
                    qTp[j * 32 : (j + 1) * 32, :],
                    1.0 / math.sqrt(DH),
                )

        # ---- x = emb + pos (per chunk); transpose x via PE ----
        # Pipelined per s-half: adds -> transposes -> xT copies -> kT mms so
        # the scores for early chunks start while the second half still loads.
        xT_sb = mp.tile([128, DC, S], f32r)
        kT_sb = mp.tile([128, DC, S], f32r)
        xTp = {}
        kTp = {}
        for dc in range(DC):
            xTp[dc] = pp.tile([128, S], f32, tag="ps512", bufs=2, name=f"xTp{dc}")
        for kc in range(DC):
            kTp[kc] = pp.tile([128, S], f32, tag="ps512", bufs=2, name=f"kTp{kc}")
        for half in range(2):
            cs = [half * 2, half * 2 + 1]
            for c in cs:
                nc.vector.tensor_add(x_sb[:, c, :], x_sb[:, c, :], pos_sb[:, c, :])
                for dc in range(DC):
                    nc.tensor.transpose(
                        xTp[dc][:, c * 128 : (c + 1) * 128],
                        x_sb[:, c, dc * 128 : (dc + 1) * 128],
                        ident,
                    )
            hs = slice(half * 256, half * 256 + 256)
            for dc in range(DC):
                nc.vector.tensor_copy(xT_sb[:, dc, hs], xTp[dc][:, hs])
            for kc in range(DC):
                for dc in range(DC):
                    nc.tensor.matmul(
                        kTp[kc][:, hs],
                        lhsT=wk_r[:, dc, kc * 128 : (kc + 1) * 128],
                        rhs=xT_sb[:, dc, hs],
                        start=(dc == 0),
                        stop=(dc == DC - 1),
                        skip_group_check=True,
                    )
                nc.vector.tensor_copy(kT_sb[:, kc, hs], kTp[kc][:, hs])


        # ---- v[s, hk] = x @ wv ----
        v_sb = mp.tile([128, SC, DE], f32r)
        for c in range(SC):
            vp = pp.tile([128, DE], f32, tag="mrg", bufs=2, name=f"vp{c}")
            for dc in range(DC):
                nc.tensor.matmul(
                    vp,
                    lhsT=xT_sb[:, dc, c * 128 : (c + 1) * 128],
                    rhs=wv_r[:, dc, :],
                    start=(dc == 0),
                    stop=(dc == DC - 1),
                )
            nc.vector.tensor_copy(v_sb[:, c, :], vp)

        # ---- scoresT[s, (h,m)] -> E = exp(scoresT); sums[(h,m)] ----
        # head h = kc*4+j lives at columns h*M of the (h,m) axis
        E_sb = mp.tile([128, SC, H * M], f32r)
        sums_p = pp.tile([1, H * M], f32, tag="sums", bufs=1)
        for c in range(SC):
            sp = pp.tile([128, H * M], f32, tag="spq", bufs=3, name=f"sp{c}")
            for kc in range(DC):
                nc.tensor.matmul(
                    sp[:, kc * 4 * M : (kc + 1) * 4 * M],
                    lhsT=kT_sb[:, kc, c * 128 : (c + 1) * 128],
                    rhs=q4_sb[:, kc, :],
                    start=True,
                    stop=True,
                    skip_group_check=True,
                )
            nc.scalar.activation(E_sb[:, c, :], sp, AF.Exp)
            nc.tensor.matmul(
                sums_p,
                lhsT=ones_col,
                rhs=E_sb[:, c, :],
                start=(c == 0),
                stop=(c == SC - 1),
                skip_group_check=True,
            )

        # ---- r = 1/sums; broadcast to all partitions ----
        r_sb = mp.tile([1, H * M], f32r)
        with nc.allow_low_precision(reason="f32r rounding of softmax recip"):
            nc.vector.reciprocal(r_sb, sums_p)
        RBp = pp.tile([128, H * M], f32, tag="spq", bufs=3, name="RBp")
        nc.tensor.matmul(
            RBp, lhsT=ones_row, rhs=r_sb, start=True, stop=True,
            skip_group_check=True,
        )
        R_sb = mp.tile([128, H * M], f32)
        for kc in range(DC):
            nc.vector.tensor_copy(
                R_sb[:, kc * 4 * M : (kc + 1) * 4 * M],
                RBp[:, kc * 4 * M : (kc + 1) * 4 * M],
            )

        # ---- ctxT[hk, m]: both kc halves into one psum bank, fused extract ----
        ctxn_sb = mp.tile([128, DC, M], f32r)
        cpB = pp.tile([128, DC, 4 * M], f32, tag="ps512", bufs=2, name="cpB")
        for kc in range(DC):
            for c in range(SC):
                nc.tensor.matmul(
                    cpB[:, kc, :],
                    lhsT=v_sb[:, c, kc * 128 : (kc + 1) * 128],
                    rhs=E_sb[:, c, kc * 4 * M : (kc + 1) * 4 * M],
                    start=(c == 0),
                    stop=(c == SC - 1),
                    skip_group_check=True,
                )
        # ctxn[j*32+dh, kc, m] = cpB[j*32+dh, kc, j*M+m] * R[j*32+dh, kc, j*M+m]
        # (R staged to SBUF: DVE cannot stream two PSUM operands)
        for j in range(4):
            nc.vector.tensor_mul(
                ctxn_sb[j * 32 : (j + 1) * 32, :, :],
                cpB[j * 32 : (j + 1) * 32, :, j * M : (j + 1) * M],
                R_sb.rearrange("p (kc hm) -> p kc hm", kc=DC)[
                    j * 32 : (j + 1) * 32, :, j * M : (j + 1) * M
                ],
            )

        # ---- ent[m, de] = ctxn.T @ wo ----
        ep = pp.tile([M, DE], f32, tag="mrg", bufs=2)
        for kc in range(DC):
            nc.tensor.matmul(
                ep,
                lhsT=ctxn_sb[:, kc, :],
                rhs=wo_r[:, kc, :],
                start=(kc == 0),
                stop=(kc == DC - 1),
            )
        ent_sb = mp.tile([M, DE], f32)
        nc.scalar.copy(ent_sb, ep)
        nc.scalar.dma_start(out=ent[:], in_=ent_sb)

    with (
        tile.TileContext(nc) as tc,
        tc.tile_pool(name="main", bufs=1) as mp,
        tc.tile_pool(name="psum", bufs=1, space="PSUM") as pp,
    ):
        _body(tc, mp, pp)

    nc.compile()
    return nc


def _get_bass():
    if "nc" not in _CACHE:
        _CACHE["nc"] = _build_bass()
    return _CACHE["nc"]


def _make_in_maps(inputs):
    emb = np.ascontiguousarray(np.asarray(inputs["embeddings"], dtype=np.float32))
    import ml_dtypes

    pos = np.ascontiguousarray(
        np.asarray(inputs["pos_enc"], dtype=np.float32)[:S].astype(ml_dtypes.bfloat16)
    )
    wq = np.ascontiguousarray(np.asarray(inputs["wq"], dtype=np.float32).reshape(D, DE))
    wkvo = np.ascontiguousarray(
        np.concatenate(
            [
                np.asarray(inputs["wk"], dtype=np.float32).reshape(D, DE),
                np.asarray(inputs["wv"], dtype=np.float32).reshape(D, DE),
                np.asarray(inputs["wo"], dtype=np.float32).reshape(DE, DE),
            ],
            axis=0,
        )
    )
    lib = np.ascontiguousarray(np.asarray(inputs["entity_library"], dtype=np.float32))
    return [
        {
            "emb": np.ascontiguousarray(emb[b]),
            "pos": pos,
            "wq": wq,
            "wkvo": wkvo,
            "lib": lib,
        }
        for b in range(N_CORES)
    ]


def run_on_device(inputs, trace=False, **kwargs):
    """Run the Bass kernel on the 8 NeuronCores; returns (entities, results)."""
    from concourse.bass_utils import run_bass_kernel_spmd

    nc = _get_bass()
    in_maps = _make_in_maps(inputs)
    res = run_bass_kernel_spmd(
        nc, in_maps, core_ids=list(range(N_CORES)), trace=trace, **kwargs
    )
    entities = np.stack([res.results[b]["ent"] for b in range(N_CORES)], axis=0)
    return entities, res


def kernel(**inputs):
    global USE_F32R
    try:
        entities, _ = run_on_device(inputs)
    except Exception:
        if not USE_F32R:
            raise
        # Fallback: rebuild with plain fp32 matmuls (4x slower on PE but
        # fully standard) in case the runtime rejects the float32r path.
        USE_F32R = False
        _CACHE.pop("nc", None)
        entities, _ = run_on_device(inputs)
    graph = np.zeros((B, M, M), dtype=np.float32)
    mask = np.zeros((B, M), dtype=np.float32)
    return entities, graph, mask

